# revision 10
# baseline (speedup 1.0000x reference)
"""Trainium2 Bass kernel for nn_DALayer (batched affine bilinear warp).

Strategy (v2):
  - 8 cores, core c handles orbits [4c, 4c+4), ALL 64 images.
  - Host builds a duplicated 4-corner table Tq fp16 [16384, 256]:
    row (y*128+x) = [X[i,y,x], X[i,y,x+1], X[i,y+1,x], X[i,y+1,x+1]] for the
    64 images i, corner-innermost.  One output position = one 512B row.
  - Device gathers one Tq row per output position with HWDGE
    indirect_dma_start (128 positions per call: one offset per partition),
    512 calls/core instead of the old 4096, descriptors 512B instead of 128B,
    and half the gather bytes (fp16).
  - VectorE: corner-weight multiply (weights broadcast across the 64-image
    axis via a 0-stride AP) + reduce over the 4-corner axis into fp32.
  - ScalarE converts fp32 -> fp16; output written as fp16 (halves write
    traffic); host converts back to fp32.  Total rel err ~1e-3 << 2e-2.
"""
import numpy as np

H = W = 128
B = 64
O = 32
NCORES = 8
OPC = O // NCORES      # orbits per core = 4
PX = H * W             # 16384
NIMG = 64
NC4 = 4
ELEM = NIMG * NC4      # fp16 per Tq row = 256 (512B)
MCALL = 32             # indirect calls per block
NPOS = 128 * MCALL     # positions per block = 4096
NJ = MCALL             # slots per partition per block
NBLK = OPC * PX // NPOS  # 16 blocks per core


def _host_coords(eps, theta_min, theta_max):
    """Replicate reference.py's coordinate math in fp32 numpy.
    Returns per-orbit flat corner base index i00 (int32 [O, PX]) and the
    4 corner weights (fp32 [O, PX, 4] ordered (y0x0, y0x1, y1x0, y1x1))."""
    f = np.float32
    thetas = (theta_min + (theta_max - theta_min) * eps).astype(f)  # [O, 7]
    ang, sx, sy, px_, py_, tx, ty = (thetas[:, i] for i in range(7))
    c, s = np.cos(ang, dtype=f), np.sin(ang, dtype=f)
    a00 = c * sx - s * py_
    a01 = c * px_ - s * sy
    a10 = s * sx + c * py_
    a11 = s * px_ + c * sy

    xg = np.linspace(-1.0, 1.0, W, dtype=f)
    yg = np.linspace(-1.0, 1.0, H, dtype=f)
    Yg, Xg = np.meshgrid(yg, xg, indexing="ij")  # [H, W]

    xs = a00[:, None, None] * Xg + a01[:, None, None] * Yg + tx[:, None, None]
    ys = a10[:, None, None] * Xg + a11[:, None, None] * Yg + ty[:, None, None]
    x = ((xs + 1.0) * ((W - 1) * 0.5)).astype(f)
    y = ((ys + 1.0) * ((H - 1) * 0.5)).astype(f)

    x0f = np.floor(x)
    y0f = np.floor(y)
    wx = x - x0f
    wy = y - y0f
    # Clamp-equivalent remap keeping x1=x0+1, y1=y0+1 (values identical to
    # reference's independent clamping because lerp is continuous):
    #   x0f < 0    -> x0=0,   fx=0
    #   x0f >= 127 -> x0=126, fx=1
    x0 = np.clip(x0f, 0, W - 2).astype(np.int64)
    y0 = np.clip(y0f, 0, H - 2).astype(np.int64)
    fx = np.where(x0f < 0, f(0), np.where(x0f >= W - 1, f(1), wx)).astype(f)
    fy = np.where(y0f < 0, f(0), np.where(y0f >= H - 1, f(1), wy)).astype(f)

    i00 = (y0 * W + x0).reshape(O, PX).astype(np.int32)
    wa = ((1 - fx) * (1 - fy)).reshape(O, PX)   # (y0, x0)
    wb = (fx * (1 - fy)).reshape(O, PX)         # (y0, x1)
    wc = ((1 - fx) * fy).reshape(O, PX)         # (y1, x0)
    wd = (fx * fy).reshape(O, PX)               # (y1, x1)
    wts = np.stack([wa, wb, wc, wd], axis=-1).astype(f)  # [O, PX, 4]
    return i00, wts


def _build_tq(X):
    """Tq fp16 [PX, ELEM]: row y*128+x = corners (y,x),(y,x+1),(y+1,x),
    (y+1,x+1) for all 64 images, corner-innermost [img, corner]."""
    Xi = np.asarray(X, np.float16).reshape(B, H, W)
    Tq = np.zeros((H, W, NIMG, NC4), np.float16)
    Tq[:, :, :, 0] = Xi.transpose(1, 2, 0)
    Tq[:, :-1, :, 1] = Xi[:, :, 1:].transpose(1, 2, 0)
    Tq[:-1, :, :, 2] = Xi[:, 1:, :].transpose(1, 2, 0)
    Tq[:-1, :-1, :, 3] = Xi[:, 1:, 1:].transpose(1, 2, 0)
    return np.ascontiguousarray(Tq.reshape(PX, ELEM))


def _build_device_program(reps=1):
    import contextlib
    import concourse.bacc as bacc
    import concourse.bass as bass
    import concourse.mybir as mybir

    nc = bacc.Bacc("TRN2")
    f16 = mybir.dt.float16
    f32 = mybir.dt.float32
    i32 = mybir.dt.int32

    table_d = nc.declare_dram_parameter("table", [PX, ELEM], f16, isOutput=False)
    idx_d = nc.declare_dram_parameter("idx", [128, NBLK * NJ], i32, isOutput=False)
    wts_d = nc.declare_dram_parameter("wts", [128, NBLK * NJ * NC4], f16, isOutput=False)
    out_d = nc.declare_dram_parameter("out", [NBLK, 128, NJ * NIMG], f16, isOutput=True)

    ctx = contextlib.ExitStack()
    with ctx:
        idx_t = ctx.enter_context(nc.sbuf_tensor("idx_t", [128, NBLK * NJ], i32))
        wts_t = ctx.enter_context(nc.sbuf_tensor("wts_t", [128, NBLK * NJ * NC4], f16))
        NBUF = 3
        g_t = [ctx.enter_context(nc.sbuf_tensor(f"g_t{i}", [128, NJ * ELEM], f16)) for i in range(NBUF)]
        u_t = [ctx.enter_context(nc.sbuf_tensor(f"u_t{i}", [128, NJ * 2 * NIMG], f16)) for i in range(NBUF)]
        acch_t = [ctx.enter_context(nc.sbuf_tensor(f"acch_t{i}", [128, NJ * NIMG], f16)) for i in range(NBUF)]
        in_sem = ctx.enter_context(nc.semaphore("in_sem"))
        gat_sem = ctx.enter_context(nc.semaphore("gat_sem"))
        cmb_sem = ctx.enter_context(nc.semaphore("cmb_sem"))
        out_sem = ctx.enter_context(nc.semaphore("out_sem"))
        block = ctx.enter_context(nc.Block())

        @block.sync
        def _(sy):
            sy.dma_start(out=idx_t[:], in_=idx_d[:]).then_inc(in_sem, 16)
            sy.dma_start(out=wts_t[:], in_=wts_d[:]).then_inc(in_sem, 16)
            for Bi in range(NBLK * reps):
                b = Bi % NBLK
                sy.wait_ge(cmb_sem, 3 * (Bi + 1))
                sy.dma_start(out=out_d[b], in_=acch_t[Bi % 3][:]).then_inc(out_sem, 16)
            sy.wait_ge(out_sem, 16 * NBLK * reps)

        @block.gpsimd
        def _(g):
            g.wait_ge(in_sem, 32)
            for Bi in range(NBLK * reps):
                b = Bi % NBLK
                p = Bi % 3
                if Bi >= 3:
                    g.wait_ge(cmb_sem, 3 * (Bi - 2))
                for m in range(MCALL):
                    g.indirect_dma_start(
                        out=g_t[p][:, m * ELEM:(m + 1) * ELEM],
                        out_offset=None,
                        in_=table_d[:],
                        in_offset=bass.IndirectOffsetOnAxis(
                            ap=idx_t[:, b * NJ + m:b * NJ + m + 1], axis=0
                        ),
                    ).then_inc(gat_sem, 16)

        @block.vector
        def _(v):
            wap = wts_t[:]
            for Bi in range(NBLK * reps):
                b = Bi % NBLK
                p = Bi % 3
                v.wait_ge(gat_sem, 16 * MCALL * (Bi + 1))
                if Bi == 0:
                    v.wait_ge(in_sem, 32)
                if Bi >= 3:
                    v.wait_ge(out_sem, 16 * (Bi - 2))
                # layout per slot j: [i0c0 i0c1 i0c2 i0c3, i1c0 ...] (c innermost)
                g4 = g_t[p][:].rearrange("q (j i c) -> q j i c", i=NIMG, c=NC4)
                t4 = g4
                w4 = bass.AP(
                    wap.tensor, wap.offset + b * NJ * NC4,
                    [wap.ap[0], [NC4, NJ], [0, NIMG], [1, NC4]],
                )
                v.tensor_tensor(out=t4, in0=g4, in1=w4, op=mybir.AluOpType.mult)
                # u[j, i, s] = t[j, i, 2s] + t[j, i, 2s+1]
                tap = g_t[p][:]
                t_ev = bass.AP(tap.tensor, tap.offset,
                               [tap.ap[0], [ELEM, NJ], [NC4, NIMG], [2, 2]])
                t_od = bass.AP(tap.tensor, tap.offset + 1,
                               [tap.ap[0], [ELEM, NJ], [NC4, NIMG], [2, 2]])
                u3 = u_t[p][:].rearrange("q (j i s) -> q j i s", i=NIMG, s=2)
                v.tensor_tensor(out=u3, in0=t_ev, in1=t_od, op=mybir.AluOpType.add)
                # acch[j, i] = u[j, i, 0] + u[j, i, 1]
                uap = u_t[p][:]
                u_ev = bass.AP(uap.tensor, uap.offset,
                               [uap.ap[0], [2 * NIMG, NJ], [2, NIMG]])
                u_od = bass.AP(uap.tensor, uap.offset + 1,
                               [uap.ap[0], [2 * NIMG, NJ], [2, NIMG]])
                v.tensor_tensor(
                    out=acch_t[p][:].rearrange("q (j i) -> q j i", i=NIMG),
                    in0=u_ev, in1=u_od, op=mybir.AluOpType.add,
                ).then_inc(cmb_sem, 3)

    nc.compile()
    return nc


_PROGRAM_CACHE = {}


def kernel(X, eps, theta_min, theta_max):
    from concourse.bass_utils import run_bass_kernel_spmd

    X = np.asarray(X, dtype=np.float32)
    eps = np.asarray(eps, dtype=np.float32)
    theta_min = np.asarray(theta_min, dtype=np.float32).reshape(-1)
    theta_max = np.asarray(theta_max, dtype=np.float32).reshape(-1)

    i00, wts = _host_coords(eps, theta_min, theta_max)  # [O,PX] i32, [O,PX,4] f32
    table = _build_tq(X)                                # [PX, 256] f16

    if "nc" not in _PROGRAM_CACHE:
        _PROGRAM_CACHE["nc"] = _build_device_program()
    nc = _PROGRAM_CACHE["nc"]

    in_maps = []
    for c in range(NCORES):
        orbs = slice(c * OPC, (c + 1) * OPC)
        # positions t = 0..OPC*PX, t = b*NPOS + j*128 + p
        idx_core = i00[orbs].reshape(NBLK, NJ, 128)          # [b, j, p]
        idx_sb = idx_core.transpose(2, 0, 1).reshape(128, NBLK * NJ)
        w_core = wts[orbs].reshape(NBLK, NJ, 128, NC4)       # [b, j, p, c]
        w_sb = w_core.transpose(2, 0, 1, 3).reshape(128, NBLK * NJ * NC4)
        in_maps.append({
            "table": table,
            "idx": np.ascontiguousarray(idx_sb),
            "wts": np.ascontiguousarray(w_sb.astype(np.float16)),
        })

    res = run_bass_kernel_spmd(nc, in_maps, list(range(NCORES)))
    _PROGRAM_CACHE["last_result"] = res

    out = np.empty((O, B, H, W), np.float32)
    for c in range(NCORES):
        o_c = res.results[c]["out"].reshape(NBLK, 128, NJ, NIMG)
        # position t = b*NPOS + j*128 + p  ->  [t, img]
        o_c = o_c.transpose(0, 2, 1, 3).reshape(OPC, PX, NIMG)
        out[c * OPC:(c + 1) * OPC] = o_c.transpose(0, 2, 1).reshape(OPC, NIMG, H, W)
    return out.reshape(O * B, H, W, 1)


# revision 13
# speedup vs baseline: 1.0338x; 1.0338x over previous
"""Trainium2 Bass kernel for nn_DALayer (batched affine bilinear warp).

Strategy (v2):
  - 8 cores, core c handles orbits [4c, 4c+4), ALL 64 images.
  - Host builds a duplicated 4-corner table Tq fp16 [16384, 256]:
    row (y*128+x) = [X[i,y,x], X[i,y,x+1], X[i,y+1,x], X[i,y+1,x+1]] for the
    64 images i, corner-innermost.  One output position = one 512B row.
  - Device gathers one Tq row per output position with HWDGE
    indirect_dma_start (128 positions per call: one offset per partition),
    512 calls/core instead of the old 4096, descriptors 512B instead of 128B,
    and half the gather bytes (fp16).
  - VectorE: corner-weight multiply (weights broadcast across the 64-image
    axis via a 0-stride AP) + reduce over the 4-corner axis into fp32.
  - ScalarE converts fp32 -> fp16; output written as fp16 (halves write
    traffic); host converts back to fp32.  Total rel err ~1e-3 << 2e-2.
"""
import numpy as np

H = W = 128
B = 64
O = 32
NCORES = 8
OPC = O // NCORES      # orbits per core = 4
PX = H * W             # 16384
NIMG = 64
NC4 = 4
ELEM = NIMG * NC4      # fp16 per Tq row = 256 (512B)
MCALL = 32             # indirect calls per block
NPOS = 128 * MCALL     # positions per block = 4096
NJ = MCALL             # slots per partition per block
NBLK = OPC * PX // NPOS  # 16 blocks per core


def _host_coords(eps, theta_min, theta_max):
    """Replicate reference.py's coordinate math in fp32 numpy.
    Returns per-orbit flat corner base index i00 (int32 [O, PX]) and the
    4 corner weights (fp32 [O, PX, 4] ordered (y0x0, y0x1, y1x0, y1x1))."""
    f = np.float32
    thetas = (theta_min + (theta_max - theta_min) * eps).astype(f)  # [O, 7]
    ang, sx, sy, px_, py_, tx, ty = (thetas[:, i] for i in range(7))
    c, s = np.cos(ang, dtype=f), np.sin(ang, dtype=f)
    a00 = c * sx - s * py_
    a01 = c * px_ - s * sy
    a10 = s * sx + c * py_
    a11 = s * px_ + c * sy

    xg = np.linspace(-1.0, 1.0, W, dtype=f)
    yg = np.linspace(-1.0, 1.0, H, dtype=f)
    Yg, Xg = np.meshgrid(yg, xg, indexing="ij")  # [H, W]

    xs = a00[:, None, None] * Xg + a01[:, None, None] * Yg + tx[:, None, None]
    ys = a10[:, None, None] * Xg + a11[:, None, None] * Yg + ty[:, None, None]
    x = ((xs + 1.0) * ((W - 1) * 0.5)).astype(f)
    y = ((ys + 1.0) * ((H - 1) * 0.5)).astype(f)

    x0f = np.floor(x)
    y0f = np.floor(y)
    wx = x - x0f
    wy = y - y0f
    # Clamp-equivalent remap keeping x1=x0+1, y1=y0+1 (values identical to
    # reference's independent clamping because lerp is continuous):
    #   x0f < 0    -> x0=0,   fx=0
    #   x0f >= 127 -> x0=126, fx=1
    x0 = np.clip(x0f, 0, W - 2).astype(np.int64)
    y0 = np.clip(y0f, 0, H - 2).astype(np.int64)
    fx = np.where(x0f < 0, f(0), np.where(x0f >= W - 1, f(1), wx)).astype(f)
    fy = np.where(y0f < 0, f(0), np.where(y0f >= H - 1, f(1), wy)).astype(f)

    i00 = (y0 * W + x0).reshape(O, PX).astype(np.int32)
    wa = ((1 - fx) * (1 - fy)).reshape(O, PX)   # (y0, x0)
    wb = (fx * (1 - fy)).reshape(O, PX)         # (y0, x1)
    wc = ((1 - fx) * fy).reshape(O, PX)         # (y1, x0)
    wd = (fx * fy).reshape(O, PX)               # (y1, x1)
    wts = np.stack([wa, wb, wc, wd], axis=-1).astype(f)  # [O, PX, 4]
    return i00, wts


def _build_tq(X):
    """Tq fp16 [PX, ELEM]: row y*128+x = [half0 | half1] where
    half0 = [X[i,y,x], X[i,y,x+1]] per image (row y), half1 = same at row
    y+1.  Layout [half, img, dx] keeps both pairwise-add inputs contiguous."""
    Xi = np.asarray(X, np.float16).reshape(B, H, W)
    Tq = np.zeros((H, W, 2, NIMG, 2), np.float16)
    Tq[:, :, 0, :, 0] = Xi.transpose(1, 2, 0)
    Tq[:, :-1, 0, :, 1] = Xi[:, :, 1:].transpose(1, 2, 0)
    Tq[:-1, :, 1, :, 0] = Xi[:, 1:, :].transpose(1, 2, 0)
    Tq[:-1, :-1, 1, :, 1] = Xi[:, 1:, 1:].transpose(1, 2, 0)
    return np.ascontiguousarray(Tq.reshape(PX, ELEM))


def _build_device_program(reps=1):
    import contextlib
    import concourse.bacc as bacc
    import concourse.bass as bass
    import concourse.mybir as mybir

    nc = bacc.Bacc("TRN2")
    f16 = mybir.dt.float16
    f32 = mybir.dt.float32
    i32 = mybir.dt.int32

    table_d = nc.declare_dram_parameter("table", [PX, ELEM], f16, isOutput=False)
    idx_d = nc.declare_dram_parameter("idx", [128, NBLK * NJ], i32, isOutput=False)
    wts_d = nc.declare_dram_parameter("wts", [128, NBLK * NJ * NC4], f16, isOutput=False)
    out_d = nc.declare_dram_parameter("out", [NBLK, 128, NJ * NIMG], f16, isOutput=True)

    ctx = contextlib.ExitStack()
    with ctx:
        idx_t = ctx.enter_context(nc.sbuf_tensor("idx_t", [128, NBLK * NJ], i32))
        wts_t = ctx.enter_context(nc.sbuf_tensor("wts_t", [128, NBLK * NJ * NC4], f16))
        NBUF = 3
        g_t = [ctx.enter_context(nc.sbuf_tensor(f"g_t{i}", [128, NJ * ELEM], f16)) for i in range(NBUF)]
        u_t = [ctx.enter_context(nc.sbuf_tensor(f"u_t{i}", [128, NJ * 2 * NIMG], f16)) for i in range(NBUF)]
        acch_t = [ctx.enter_context(nc.sbuf_tensor(f"acch_t{i}", [128, NJ * NIMG], f16)) for i in range(NBUF)]
        in_sem = ctx.enter_context(nc.semaphore("in_sem"))
        gat_sem = ctx.enter_context(nc.semaphore("gat_sem"))
        cmb_sem = ctx.enter_context(nc.semaphore("cmb_sem"))
        out_sem = ctx.enter_context(nc.semaphore("out_sem"))
        block = ctx.enter_context(nc.Block())

        @block.sync
        def _(sy):
            sy.dma_start(out=idx_t[:], in_=idx_d[:]).then_inc(in_sem, 16)
            sy.dma_start(out=wts_t[:], in_=wts_d[:]).then_inc(in_sem, 16)
            for Bi in range(NBLK * reps):
                b = Bi % NBLK
                sy.wait_ge(cmb_sem, 3 * (Bi + 1))
                sy.dma_start(out=out_d[b], in_=acch_t[Bi % 3][:]).then_inc(out_sem, 16)
            sy.wait_ge(out_sem, 16 * NBLK * reps)

        @block.gpsimd
        def _(g):
            g.wait_ge(in_sem, 32)
            for Bi in range(NBLK * reps):
                b = Bi % NBLK
                p = Bi % 3
                if Bi >= 3:
                    g.wait_ge(cmb_sem, 3 * (Bi - 2))
                for m in range(MCALL):
                    g.indirect_dma_start(
                        out=g_t[p][:, m * ELEM:(m + 1) * ELEM],
                        out_offset=None,
                        in_=table_d[:],
                        in_offset=bass.IndirectOffsetOnAxis(
                            ap=idx_t[:, b * NJ + m:b * NJ + m + 1], axis=0
                        ),
                    ).then_inc(gat_sem, 16)

        @block.vector
        def _(v):
            wap = wts_t[:]
            for Bi in range(NBLK * reps):
                b = Bi % NBLK
                p = Bi % 3
                v.wait_ge(gat_sem, 16 * MCALL * (Bi + 1))
                if Bi == 0:
                    v.wait_ge(in_sem, 32)
                if Bi >= 3:
                    v.wait_ge(out_sem, 16 * (Bi - 2))
                # layout per slot j: [half(2), img(64), dx(2)] = 256 fp16;
                # (j, half) merged: g stride 128, w stride 2
                gap = g_t[p][:]
                g4 = bass.AP(gap.tensor, gap.offset,
                             [gap.ap[0], [2 * NIMG, 2 * NJ], [2, NIMG], [1, 2]])
                w4 = bass.AP(
                    wap.tensor, wap.offset + b * NJ * NC4,
                    [wap.ap[0], [2, 2 * NJ], [0, NIMG], [1, 2]],
                )
                v.tensor_tensor(out=g4, in0=g4, in1=w4, op=mybir.AluOpType.mult)
                # u[j, i, d] = t[j, 0, i, d] + t[j, 1, i, d]  (contiguous halves)
                tap = g_t[p][:]
                t_h0 = bass.AP(tap.tensor, tap.offset,
                               [tap.ap[0], [ELEM, NJ], [1, 2 * NIMG]])
                t_h1 = bass.AP(tap.tensor, tap.offset + 2 * NIMG,
                               [tap.ap[0], [ELEM, NJ], [1, 2 * NIMG]])
                u3 = u_t[p][:].rearrange("q (j e) -> q j e", e=2 * NIMG)
                v.tensor_tensor(out=u3, in0=t_h0, in1=t_h1, op=mybir.AluOpType.add)
                # acch[j, i] = u[j, i, 0] + u[j, i, 1]
                uap = u_t[p][:]
                u_ev = bass.AP(uap.tensor, uap.offset,
                               [uap.ap[0], [2 * NIMG, NJ], [2, NIMG]])
                u_od = bass.AP(uap.tensor, uap.offset + 1,
                               [uap.ap[0], [2 * NIMG, NJ], [2, NIMG]])
                v.tensor_tensor(
                    out=acch_t[p][:].rearrange("q (j i) -> q j i", i=NIMG),
                    in0=u_ev, in1=u_od, op=mybir.AluOpType.add,
                ).then_inc(cmb_sem, 3)

    nc.compile()
    return nc


_PROGRAM_CACHE = {}


def kernel(X, eps, theta_min, theta_max):
    from concourse.bass_utils import run_bass_kernel_spmd

    X = np.asarray(X, dtype=np.float32)
    eps = np.asarray(eps, dtype=np.float32)
    theta_min = np.asarray(theta_min, dtype=np.float32).reshape(-1)
    theta_max = np.asarray(theta_max, dtype=np.float32).reshape(-1)

    i00, wts = _host_coords(eps, theta_min, theta_max)  # [O,PX] i32, [O,PX,4] f32
    table = _build_tq(X)                                # [PX, 256] f16

    if "nc" not in _PROGRAM_CACHE:
        _PROGRAM_CACHE["nc"] = _build_device_program()
    nc = _PROGRAM_CACHE["nc"]

    in_maps = []
    for c in range(NCORES):
        orbs = slice(c * OPC, (c + 1) * OPC)
        # positions t = 0..OPC*PX, t = b*NPOS + j*128 + p
        idx_core = i00[orbs].reshape(NBLK, NJ, 128)          # [b, j, p]
        idx_sb = idx_core.transpose(2, 0, 1).reshape(128, NBLK * NJ)
        w_core = wts[orbs].reshape(NBLK, NJ, 128, NC4)       # [b, j, p, c]
        w_sb = w_core.transpose(2, 0, 1, 3).reshape(128, NBLK * NJ * NC4)
        in_maps.append({
            "table": table,
            "idx": np.ascontiguousarray(idx_sb),
            "wts": np.ascontiguousarray(w_sb.astype(np.float16)),
        })

    res = run_bass_kernel_spmd(nc, in_maps, list(range(NCORES)))
    _PROGRAM_CACHE["last_result"] = res

    out = np.empty((O, B, H, W), np.float32)
    for c in range(NCORES):
        o_c = res.results[c]["out"].reshape(NBLK, 128, NJ, NIMG)
        # position t = b*NPOS + j*128 + p  ->  [t, img]
        o_c = o_c.transpose(0, 2, 1, 3).reshape(OPC, PX, NIMG)
        out[c * OPC:(c + 1) * OPC] = o_c.transpose(0, 2, 1).reshape(OPC, NIMG, H, W)
    return out.reshape(O * B, H, W, 1)
